# revision 17
# baseline (speedup 1.0000x reference)
"""Trainium2 Bass kernel for a 2-layer GCN (PyG GCNConv semantics).

    out = Ahat @ relu(Ahat @ (X W1) + b1) @ W2 + b2,  Ahat = D^-1/2 (A+I) D^-1/2

Math restructure: norm(e) = dinv[src]*dinv[dst] is separable and the
aggregation commutes with the dense matmul, so with xs = dinv ⊙ X (scaled on
the host) layer 1 is

    h = relu(dinv ⊙ ((segsum xs[src]) @ W1) + b1)

i.e. gather + segment-sum FIRST (pre-W1, 128-wide bf16 rows), then one
128x128 matmul per 128-dest window. There is no dense per-node pass at all.

Gathers use the bulk SWDGE dma_gather (Q7-generated descriptors, int16
indices). int16 reach is 32768 rows, the table has 50177, so the table is
addressed through two overlapping views (A = rows [0, 32768), B = rows
[17409, 50177)); nodes are permuted into table rows so high-out-degree
sources land in the overlap (flexible per-slot A/B choice) and the rest are
split by a greedy per-dest balance. Each 128-dest window gets global A/B
round budgets; slots pad with zero rows (row 0 / row TROWS-1).

Distribution (8 NeuronCores, SPMD): destinations sharded round-robin over
degree-sorted order; each core owns 49 windows of 128 dests. TensorE
identity-matmuls accumulate gathered rounds into PSUM f32. Two SPMD
dispatches: P1 emits u2 = dinv ⊙ (relu(h) W2) shards (bf16); the host
scatters them into the full u2 table (pure data staging) and P2 aggregates
layer 2 identically (table rows padded to 128 bf16). Weights replicated.
"""

import os
from contextlib import ExitStack

import ml_dtypes
import numpy as np

N, E, IN, HID, OUT = 50000, 600000, 128, 128, 64
NCORE = 8
P = 128
NPADN = 50176  # nodes padded to 392 tiles of 128
DPC = 6272  # dests per core (49 windows * 128)
NW = DPC // P  # 49 windows
TROWS = 1 + NPADN  # table rows: row 0 = zeros, row 1+t = permuted node t
BF16 = ml_dtypes.bfloat16

SPLIT = 32768  # int16 index reach (rows)
VB0 = TROWS - SPLIT  # 17409; view B = table rows [VB0, TROWS)
NOVER = SPLIT - VB0  # flex rows [VB0, SPLIT)
NA_ONLY = VB0 - 1  # forced-A rows [1, VB0)
NPADS = NPADN - N
BPAD = SPLIT - 1  # B-view index of row TROWS-1 (zeros)

CAPR = int(os.environ.get("GCN_CAPR", "8"))  # rounds per dma_gather (1024 idx cap)
GBUFS = int(os.environ.get("GCN_GBUFS", "6"))  # staged chunk buffers per stream

_CACHE = {}


# ---------------------------------------------------------------- host prep


def _prep(edge_index):
    row = np.asarray(edge_index[0], dtype=np.int64)
    col = np.asarray(edge_index[1], dtype=np.int64)
    deg = np.bincount(col, minlength=N) + 1  # in-degree + self
    dinv = np.zeros(NPADN, np.float32)
    dinv[:N] = (1.0 / np.sqrt(deg.astype(np.float64))).astype(np.float32)

    # shard dests: degree-sorted, dealt round-robin so core profiles match
    order = np.argsort(-deg, kind="stable")
    dests = np.full((NCORE, DPC), -1, np.int64)
    for c in range(NCORE):
        mine = order[c::NCORE]
        dests[c, : len(mine)] = mine

    dinv_win = np.zeros((NCORE, P, NW), np.float32)
    for c in range(NCORE):
        d = dests[c]
        dv = np.where(d >= 0, dinv[np.clip(d, 0, N - 1)], 0.0).astype(np.float32)
        dinv_win[c] = dv.reshape(NW, P).T

    # ---- node -> table row permutation: high fan-out sources into the
    # overlap region; the rest split A/B by greedy per-dest balance.
    src_slots = np.bincount(row, minlength=N) + 1
    rank = np.argsort(-src_slots, kind="stable")
    over_nodes = rank[:NOVER]
    rest = rank[NOVER:]

    rorder = np.argsort(row, kind="stable")
    dsts_by_src = col[rorder]
    rcnt = np.bincount(row, minlength=N)
    rstarts = np.concatenate([[0], np.cumsum(rcnt)])[:N]

    NB_ONLY = TROWS - SPLIT - NPADS
    imb = np.zeros(N, np.int32)  # per-dest forced (#A - #B)
    sideA = np.zeros(len(rest), bool)
    nA = nB = 0
    for i, v in enumerate(rest):
        D = dsts_by_src[rstarts[v] : rstarts[v] + rcnt[v]]
        s = int(imb[D].sum()) + int(imb[v])
        if nA < NA_ONLY and (s <= 0 or nB >= NB_ONLY):
            sideA[i] = True
            nA += 1
            imb[D] += 1
            imb[v] += 1
        else:
            nB += 1
            imb[D] -= 1
            imb[v] -= 1
    a_nodes = rest[sideA]
    b_nodes = rest[~sideA]

    rowof = np.zeros(NPADN, np.int64)
    rowof[a_nodes] = 1 + np.arange(len(a_nodes))
    rowof[over_nodes] = VB0 + np.arange(NOVER)
    rowof[b_nodes] = SPLIT + np.arange(len(b_nodes))
    rowof[N:] = SPLIT + len(b_nodes) + np.arange(NPADS)  # pads end at TROWS-1

    # ---- per-dest slot tables (table rows) per core
    eorder = np.argsort(col, kind="stable")
    srcs_sorted = row[eorder]
    cnt = np.bincount(col, minlength=N)
    starts = np.concatenate([[0], np.cumsum(cnt)])[:N]

    R0 = int(deg[dests[dests >= 0]].max())
    arr_all = np.zeros((NCORE, DPC, R0), np.int64)
    key_all = np.full((NCORE, DPC, R0), 3, np.int8)  # 0=A,1=flex,2=B,3=inval
    for c in range(NCORE):
        d = dests[c]
        dcnt = np.where(d >= 0, deg[np.clip(d, 0, N - 1)], 0)
        dstart = np.where(d >= 0, starts[np.clip(d, 0, N - 1)], 0)
        rr = np.arange(R0)[None, :]
        gpos = np.clip(dstart[:, None] + rr - 1, 0, E - 1)
        er = rowof[srcs_sorted[gpos]]
        valid = (rr >= 1) & (rr < dcnt[:, None])
        arr = np.where(valid, er, 0)
        arr[:, 0] = np.where(d >= 0, rowof[np.clip(d, 0, N - 1)], 0)
        valid[:, 0] = d >= 0
        arr_all[c] = arr
        key_all[c] = np.where(
            ~valid, 3, np.where(arr < VB0, 0, np.where(arr < SPLIT, 1, 2))
        )

    fA = (key_all == 0).sum(2)
    fx = (key_all == 1).sum(2)
    fB = (key_all == 2).sum(2)
    degm = fA + fx + fB

    # ---- per-window A/B round budgets (shared across cores)
    A_ws = np.zeros(NW, np.int64)
    B_ws = np.zeros(NW, np.int64)
    for w in range(NW):
        sl = slice(w * P, (w + 1) * P)
        wfA, wfx = fA[:, sl].ravel(), fx[:, sl].ravel()
        wfB, wdeg = fB[:, sl].ravel(), degm[:, sl].ravel()
        amin = int(wfA.max())
        amax = int(np.minimum(wfA + wfx, wdeg).max())
        best = None
        for A in range(amin, amax + 1):
            B = int(np.maximum(wfB, wdeg - np.minimum(wfA + wfx, A)).max())
            if best is None or A + B < best[0]:
                best = (A + B, A, B)
        A_ws[w], B_ws[w] = best[1], best[2]

    offsA = np.concatenate([[0], np.cumsum(A_ws)])
    offsB = np.concatenate([[0], np.cumsum(B_ws)])
    TA, TB = int(offsA[-1]), int(offsB[-1])

    # ---- slot assignment + idx16 streams
    idxA = np.zeros((NCORE, 16, TA * 8), np.int16)
    idxB = np.full((NCORE, 16, TB * 8), BPAD, np.int16)
    for c in range(NCORE):
        ordr = np.argsort(key_all[c], axis=1, kind="stable")
        srows = np.take_along_axis(arr_all[c], ordr, axis=1)  # [DPC, R0]
        Bw_d = np.repeat(B_ws, P)
        a_d = np.maximum(fA[c], degm[c] - Bw_d)
        b_d = degm[c] - a_d
        for w in range(NW):
            sl = slice(w * P, (w + 1) * P)
            Aw, Bw = int(A_ws[w]), int(B_ws[w])
            if Aw:
                jj = np.arange(Aw)[None, :]
                blk = np.where(
                    jj < a_d[sl][:, None],
                    np.take_along_axis(
                        srows[sl], np.minimum(jj, R0 - 1).repeat(P, 0), axis=1
                    )[:, :Aw],
                    0,
                )  # [P, Aw] table rows; pad row 0
                nidx = Aw * P
                j = np.arange(nidx)
                idxA[c][j % 16, int(offsA[w]) * 8 + j // 16] = blk[
                    j % P, j // P
                ].astype(np.int16)
            if Bw:
                jj = np.arange(Bw)[None, :]
                gidx = np.minimum(a_d[sl][:, None] + jj, R0 - 1)
                blk = np.where(
                    jj < b_d[sl][:, None],
                    np.take_along_axis(srows[sl], gidx, axis=1) - VB0,
                    BPAD,
                )  # [P, Bw] B-view indices; pad BPAD (zeros row)
                nidx = Bw * P
                j = np.arange(nidx)
                idxB[c][j % 16, int(offsB[w]) * 8 + j // 16] = blk[
                    j % P, j // P
                ].astype(np.int16)

    return {
        "dinv": dinv,
        "dests": dests,
        "rowof": rowof,
        "dinv_win": dinv_win,
        "A_ws": tuple(int(a) for a in A_ws),
        "B_ws": tuple(int(b) for b in B_ws),
        "offsA": offsA,
        "offsB": offsB,
        "TA": TA,
        "TB": TB,
        "idxA": np.tile(idxA, (1, 8, 1)),  # replicate over Q7 cores -> [128, .]
        "idxB": np.tile(idxB, (1, 8, 1)),
    }


# ------------------------------------------------------------- bass builders


def _new_nc():
    import concourse.bacc as bacc

    return bacc.Bacc("TRN2", target_bir_lowering=False, debug=False, num_devices=NCORE)


class _Stream:
    """Chunked dma_gather stream over a flat per-phase round sequence.

    The stream's T rounds are cut into CAPR-round chunks (<= 1024 indices per
    dma_gather — the SWDGE descriptor-ring carveout limit). Windows consume
    rounds across chunk boundaries via rhs(); chunks are emitted just-in-time
    and pipeline through the tile pool's rotating buffers.
    """

    def __init__(self, nc, pool, tag, table_view, idx_sb, T, felem, dt):
        self.nc, self.pool, self.tag = nc, pool, tag
        self.table_view, self.idx_sb = table_view, idx_sb
        self.T, self.felem, self.dt = T, felem, dt
        self.tiles = []

    def ensure(self, upto_round):
        while len(self.tiles) * CAPR < min(upto_round, self.T):
            i = len(self.tiles)
            c0, c1 = i * CAPR, min((i + 1) * CAPR, self.T)
            rg = c1 - c0
            t = self.pool.tile(
                [P, rg * self.felem], self.dt, tag=self.tag, name=f"{self.tag}{i}"
            )
            self.nc.gpsimd.dma_gather(
                t[:].rearrange("p (r f) -> p r f", r=rg),
                self.table_view,
                self.idx_sb[:, c0 * 8 : c1 * 8],
                rg * P,
                rg * P,
                self.felem,
            )
            self.tiles.append(t)

    def rhs(self, r):
        i, o = divmod(r, CAPR)
        return self.tiles[i][:, o * self.felem : (o + 1) * self.felem]


def _accum_window(nc, ident, acc, sA, sB, w, prep):
    """Identity-matmul accumulate the window's A+B rounds into PSUM acc."""
    offsA, offsB = prep["offsA"], prep["offsB"]
    na, nb = prep["A_ws"][w], prep["B_ws"][w]
    tot = na + nb
    k = 0
    for r in range(int(offsA[w]), int(offsA[w]) + na):
        nc.tensor.matmul(
            out=acc[:], lhsT=ident[:], rhs=sA.rhs(r),
            start=(k == 0), stop=(k == tot - 1),
        )
        k += 1
    for r in range(int(offsB[w]), int(offsB[w]) + nb):
        nc.tensor.matmul(
            out=acc[:], lhsT=ident[:], rhs=sB.rhs(r),
            start=(k == 0), stop=(k == tot - 1),
        )
        k += 1


def _build_p1(prep, nrep=None):
    import concourse.tile as tile
    from concourse import mybir
    from concourse.masks import make_identity
    from concourse.library_config import mlp

    nc = _new_nc()
    A_ws, B_ws = prep["A_ws"], prep["B_ws"]
    offsA, offsB = prep["offsA"], prep["offsB"]
    TA, TB = prep["TA"], prep["TB"]
    f32, bf16, i16 = mybir.dt.float32, mybir.dt.bfloat16, mybir.dt.int16
    xs = nc.declare_dram_parameter("xs", [TROWS, IN], bf16, isOutput=False)
    W1m = nc.declare_dram_parameter("W1m", [IN, HID], bf16, isOutput=False)
    W2m = nc.declare_dram_parameter("W2m", [HID, OUT], bf16, isOutput=False)
    b1t = nc.declare_dram_parameter("b1t", [P, HID], f32, isOutput=False)
    idxAp = nc.declare_dram_parameter("idxA", [P, TA * 8], i16, isOutput=False)
    idxBp = nc.declare_dram_parameter("idxB", [P, TB * 8], i16, isOutput=False)
    dinv_w = nc.declare_dram_parameter("dinv_w", [P, NW], f32, isOutput=False)
    u2s = nc.declare_dram_parameter("u2s", [DPC, OUT], bf16, isOutput=True)

    with tile.TileContext(nc) as tc, ExitStack() as ctx:
        cpool = ctx.enter_context(tc.tile_pool(name="const", bufs=1))
        gpool = ctx.enter_context(tc.tile_pool(name="gath", bufs=GBUFS))
        bpool = ctx.enter_context(tc.tile_pool(name="work", bufs=3))
        q1 = ctx.enter_context(tc.tile_pool(name="ps_accx", bufs=2, space="PSUM"))
        q2 = ctx.enter_context(tc.tile_pool(name="ps_T", bufs=2, space="PSUM"))
        q3 = ctx.enter_context(tc.tile_pool(name="ps_h", bufs=2, space="PSUM"))
        q5 = ctx.enter_context(tc.tile_pool(name="ps_u2", bufs=2, space="PSUM"))

        nc.gpsimd.load_library(mlp)

        identB = cpool.tile([P, P], bf16)
        make_identity(nc, identB[:])
        w1sb = cpool.tile([IN, HID], bf16)
        nc.sync.dma_start(out=w1sb[:], in_=W1m[:])
        w2sb = cpool.tile([HID, OUT], bf16)
        nc.sync.dma_start(out=w2sb[:], in_=W2m[:])
        b1sb = cpool.tile([P, HID], f32)
        nc.sync.dma_start(out=b1sb[:], in_=b1t[:])
        idxA_sb = cpool.tile([P, TA * 8], i16)
        nc.sync.dma_start(out=idxA_sb[:], in_=idxAp[:])
        idxB_sb = cpool.tile([P, TB * 8], i16)
        nc.sync.dma_start(out=idxB_sb[:], in_=idxBp[:])
        dw_sb = cpool.tile([P, NW], f32)
        nc.sync.dma_start(out=dw_sb[:], in_=dinv_w[:])

        rep = tc.For_i(0, nrep, 1) if nrep else None
        if rep is not None:
            rep.__enter__()

        sA = _Stream(nc, gpool, "sA", xs[0:SPLIT, :], idxA_sb, TA, IN, bf16)
        sB = _Stream(nc, gpool, "sB", xs[VB0:TROWS, :], idxB_sb, TB, IN, bf16)
        for w in range(NW):
            wpf = min(w + 2, NW)  # prefetch two windows ahead
            sA.ensure(int(offsA[wpf]))
            sB.ensure(int(offsB[wpf]))
            accx = q1.tile([P, IN], f32, space="PSUM")
            _accum_window(nc, identB, accx, sA, sB, w, prep)
            # aggx @ W1 needs aggx^T as lhsT: PSUM->SBUF, transpose, copy
            accs = bpool.tile([P, IN], bf16, tag="accs")
            nc.scalar.copy(accs[:], accx[:])
            psT = q2.tile([P, P], bf16, space="PSUM", tag="psT", bufs=1)
            nc.tensor.transpose(out=psT[:], in_=accs[:], identity=identB[:])
            accT = bpool.tile([P, P], bf16, tag="accT")
            nc.vector.tensor_copy(accT[:], psT[:])
            ps = q3.tile([P, HID], f32, space="PSUM")
            nc.tensor.matmul(
                out=ps[:], lhsT=accT[:], rhs=w1sb[:], start=True, stop=True
            )
            # h = relu(dinv_d * ps + b1)
            m1 = bpool.tile([P, HID], f32, tag="m1")
            nc.scalar.activation(
                out=m1[:], in_=ps[:],
                func=mybir.ActivationFunctionType.Copy,
                scale=dw_sb[:, w : w + 1],
            )
            m2 = bpool.tile([P, HID], f32, tag="m2")
            nc.vector.tensor_add(m2[:], m1[:], b1sb[:])
            hw = bpool.tile([P, HID], bf16, tag="hw")
            nc.vector.tensor_scalar_max(hw[:], m2[:], 0.0)
            # u2 = dinv_d * (h @ W2)
            psT2 = q2.tile([P, P], bf16, space="PSUM", tag="psT2", bufs=1)
            nc.tensor.transpose(out=psT2[:], in_=hw[:], identity=identB[:])
            hT = bpool.tile([P, P], bf16, tag="hT")
            nc.vector.tensor_copy(hT[:], psT2[:])
            ps3 = q5.tile([P, OUT], f32, space="PSUM")
            nc.tensor.matmul(
                out=ps3[:], lhsT=hT[:], rhs=w2sb[:], start=True, stop=True
            )
            u2t = bpool.tile([P, OUT], bf16, tag="u2t")
            nc.scalar.activation(
                out=u2t[:], in_=ps3[:],
                func=mybir.ActivationFunctionType.Copy,
                scale=dw_sb[:, w : w + 1],
            )
            nc.sync.dma_start(out=u2s[w * P : (w + 1) * P, :], in_=u2t[:])

        if rep is not None:
            rep.__exit__(None, None, None)

    nc.compile()
    return nc


def _build_p2(prep, nrep=None):
    import concourse.tile as tile
    from concourse import mybir
    from concourse.masks import make_identity
    from concourse.library_config import mlp

    nc = _new_nc()
    A_ws, B_ws = prep["A_ws"], prep["B_ws"]
    offsA, offsB = prep["offsA"], prep["offsB"]
    TA, TB = prep["TA"], prep["TB"]
    f32, bf16, i16 = mybir.dt.float32, mybir.dt.bfloat16, mybir.dt.int16
    u2b = nc.declare_dram_parameter("u2b", [TROWS, HID], bf16, isOutput=False)
    idxAp = nc.declare_dram_parameter("idxA", [P, TA * 8], i16, isOutput=False)
    idxBp = nc.declare_dram_parameter("idxB", [P, TB * 8], i16, isOutput=False)
    dinv_w = nc.declare_dram_parameter("dinv_w", [P, NW], f32, isOutput=False)
    b2t = nc.declare_dram_parameter("b2t", [P, OUT], f32, isOutput=False)
    outs = nc.declare_dram_parameter("outs", [DPC, OUT], f32, isOutput=True)

    with tile.TileContext(nc) as tc, ExitStack() as ctx:
        cpool = ctx.enter_context(tc.tile_pool(name="const", bufs=1))
        gpool = ctx.enter_context(tc.tile_pool(name="gath", bufs=GBUFS))
        bpool = ctx.enter_context(tc.tile_pool(name="work", bufs=3))
        qpool = ctx.enter_context(tc.tile_pool(name="psum", bufs=3, space="PSUM"))

        nc.gpsimd.load_library(mlp)

        identB = cpool.tile([P, P], bf16)
        make_identity(nc, identB[:])
        idxA_sb = cpool.tile([P, TA * 8], i16)
        nc.sync.dma_start(out=idxA_sb[:], in_=idxAp[:])
        idxB_sb = cpool.tile([P, TB * 8], i16)
        nc.sync.dma_start(out=idxB_sb[:], in_=idxBp[:])
        dw_sb = cpool.tile([P, NW], f32)
        nc.sync.dma_start(out=dw_sb[:], in_=dinv_w[:])
        b2sb = cpool.tile([P, OUT], f32)
        nc.sync.dma_start(out=b2sb[:], in_=b2t[:])

        rep = tc.For_i(0, nrep, 1) if nrep else None
        if rep is not None:
            rep.__enter__()

        sA = _Stream(nc, gpool, "sA", u2b[0:SPLIT, :], idxA_sb, TA, HID, bf16)
        sB = _Stream(nc, gpool, "sB", u2b[VB0:TROWS, :], idxB_sb, TB, HID, bf16)
        for w in range(NW):
            wpf = min(w + 2, NW)
            sA.ensure(int(offsA[wpf]))
            sB.ensure(int(offsB[wpf]))
            acc = qpool.tile([P, HID], f32, space="PSUM")
            _accum_window(nc, identB, acc, sA, sB, w, prep)
            m1 = bpool.tile([P, OUT], f32, tag="m1")
            nc.scalar.activation(
                out=m1[:], in_=acc[:, 0:OUT],
                func=mybir.ActivationFunctionType.Copy,
                scale=dw_sb[:, w : w + 1],
            )
            o = bpool.tile([P, OUT], f32, tag="o")
            nc.vector.tensor_add(o[:], m1[:], b2sb[:])
            nc.sync.dma_start(out=outs[w * P : (w + 1) * P, :], in_=o[:])

        if rep is not None:
            rep.__exit__(None, None, None)

    nc.compile()
    return nc


# ------------------------------------------------------------------- driver


def _nrep():
    v = os.environ.get("GCN_NREP", "")
    return int(v) if v else None


def kernel(x, edge_index, W1, b1, W2, b2):
    from concourse.bass_utils import run_bass_kernel_spmd

    x = np.asarray(x, np.float32)
    W1 = np.asarray(W1, np.float32)
    b1 = np.asarray(b1, np.float32)
    W2 = np.asarray(W2, np.float32)
    b2 = np.asarray(b2, np.float32)

    prep = _prep(edge_index)
    key = (prep["TA"], prep["TB"], CAPR, _nrep())
    if key not in _CACHE:
        _CACHE[key] = (_build_p1(prep, _nrep()), _build_p2(prep, _nrep()))
    nc1, nc2 = _CACHE[key]

    # xs table: row 0 zeros, row rowof[s] = dinv[s] * x[s] (bf16)
    xs = np.zeros((TROWS, IN), np.float32)
    xs[prep["rowof"][:N]] = x * prep["dinv"][:N, None]
    xs = xs.astype(BF16)
    b1t = np.broadcast_to(b1[None, :], (P, HID)).copy()
    b2t = np.broadcast_to(b2[None, :], (P, OUT)).copy()

    core_ids = list(range(NCORE))
    in1 = [
        {
            "xs": xs,
            "W1m": W1.astype(BF16),
            "W2m": W2.astype(BF16),
            "b1t": b1t,
            "idxA": prep["idxA"][c],
            "idxB": prep["idxB"][c],
            "dinv_w": prep["dinv_win"][c],
        }
        for c in core_ids
    ]
    res1 = run_bass_kernel_spmd(nc1, in1, core_ids)

    # host staging: scatter u2 shards into the padded table (data movement)
    u2b = np.zeros((TROWS, HID), BF16)
    for c in core_ids:
        d = prep["dests"][c]
        v = d >= 0
        u2b[prep["rowof"][d[v]], :OUT] = res1.results[c]["u2s"][v]

    in2 = [
        {
            "u2b": u2b,
            "idxA": prep["idxA"][c],
            "idxB": prep["idxB"][c],
            "dinv_w": prep["dinv_win"][c],
            "b2t": b2t,
        }
        for c in core_ids
    ]
    res2 = run_bass_kernel_spmd(nc2, in2, core_ids)

    out = np.zeros((N, OUT), np.float32)
    for c in core_ids:
        d = prep["dests"][c]
        v = d >= 0
        out[d[v]] = res2.results[c]["outs"][v]
    return out
